# revision 45
# baseline (speedup 1.0000x reference)
"""Trainium2 Bass kernel for nn_AttentionLayer_79293686218841 — v3.

Sharding: 8 cores = 4 batches x 2 query-halves; each core runs all 8 heads
for its 1024 queries against all 2048 keys.

p-stationary attention: per (head, 128-query block, 128-key chunk) the
exp'd score chunk p [128k, 128q] is the matmul stationary operand and
V65 = [V | mask] the moving operand, so OV accumulates as [q, 65] with the
softmax denominator in column 64 on the query's own partition.  The
masked-query uniform fallback is a contraction-1 "virtual key" matmul
(mqi x meanrow) into the same PSUM tile.  Normalization + transpose back
to [d, q] happens in one matmul against diag(1/den).

v3 performance structure (the exp drain is the co-bottleneck with PE):
- softmax exp is SPLIT between the ACT engine (table exp) and the DVE
  (Schraudolph-style int16 bit-trick writing bf16 bit patterns), ~10/6
  per 16-chunk block, sized so both engines carry ~equal PSUM-drain load;
- x arrives pre-transposed from host marshaling (no on-device transpose
  DMAs) and the x-mean for the masked-query fallback is precomputed on
  host, split across three DMA queues (SP/ACT/Pool);
- K/Q PSUM drains alternate ACT/DVE; SBUF-side scales/copies run on the
  otherwise-idle GPSIMD;
- the it2=1 output projection is split 3/4 + 1/4 so only the last quarter
  of the contraction waits on the final attention block.
"""

import sys

try:
    import concourse.bass  # noqa: F401
except ImportError:
    sys.path.insert(0, "/opt/trn_rl_repo")


import numpy as np
import ml_dtypes

import concourse.bass as bass
import concourse.mybir as mybir
import concourse.tile as tile
from concourse import bacc

f32 = mybir.dt.float32
bf16 = mybir.dt.bfloat16
i16 = mybir.dt.int16
AF = mybir.ActivationFunctionType
OP = mybir.AluOpType

B, N, D = 4, 2048, 512
H, DH = 8, 64
NI = 1024      # queries per core
P = 128
NJT = N // P   # 16 key chunks

# DVE bit-trick exp: exp(s*0.125) ~= bf16_bits(int16(s*EXP_A + EXP_B)).
# Constructs the bf16 bit pattern directly: exponent from floor(y*log2e),
# mantissa from the linear Schraudolph approximation of 2^frac.  Max ~4%
# per-element error; softmax normalization cancels the common mode, final
# output error ~1e-2 (measured).  big_neg never reaches s (masks are folded
# into V rows / the v65 mask column instead), so no clamping is needed.
EXP_A = 0.125 * 1.4426950408889634 * 128.0
EXP_C = 4.8
EXP_B = 127.0 * 128.0 - EXP_C
# jt chunks whose exp runs on DVE (bit-trick) instead of ACT, per block
DVE_EXP_JTS = frozenset({2, 4, 7, 9, 11, 14})


def build_nc(reps: int = 1, dve_jts=None):
    dve_exp_jts = DVE_EXP_JTS if dve_jts is None else frozenset(dve_jts)
    nc = bacc.Bacc("TRN2", target_bir_lowering=False, debug=False, num_devices=8)

    # x / xq arrive pre-transposed from host marshaling: [p, d_chunk, n]
    xT_d = nc.dram_tensor("xT", [P, 4, N], bf16, kind="ExternalInput")
    xqT_d = nc.dram_tensor("xqT", [P, 4, NI], bf16, kind="ExternalInput")
    mk_d = nc.dram_tensor("mk", [P, NJT], f32, kind="ExternalInput")
    mx_d = nc.dram_tensor("mx", [P, 4], bf16, kind="ExternalInput")
    mqi_d = nc.dram_tensor("mqi", [NI], bf16, kind="ExternalInput")
    id_d = nc.dram_tensor("ident", [P, P], bf16, kind="ExternalInput")
    wq_d = nc.dram_tensor("Wq", [D, D], bf16, kind="ExternalInput")
    wk_d = nc.dram_tensor("Wk", [D, D], bf16, kind="ExternalInput")
    wv_d = nc.dram_tensor("Wv", [D, D], bf16, kind="ExternalInput")
    wo_d = nc.dram_tensor("Wo", [D, D], bf16, kind="ExternalInput")
    bo_d = nc.dram_tensor("bo", [D], f32, kind="ExternalInput")
    out_d = nc.dram_tensor("out", [NI, D], f32, kind="ExternalOutput")
    # dummy input whose shape depends on `reps`: defeats the weak NEFF-cache
    # key (which hashes only the wrapper HLO, not the embedded BIR)
    nc.dram_tensor("pad", [reps, 2], f32, kind="ExternalInput")

    with tile.TileContext(nc) as tc:
        with (
            tc.tile_pool(name="cw", bufs=1) as cw,
            tc.tile_pool(name="cst", bufs=1) as cst,
            tc.tile_pool(name="xp", bufs=1) as xp,
            tc.tile_pool(name="kqv", bufs=1) as kqv,
            tc.tile_pool(name="pT", bufs=4) as pTp,
            tc.tile_pool(name="obf", bufs=4) as obf,
            tc.tile_pool(name="dg", bufs=4) as dgp,
            tc.tile_pool(name="rows", bufs=4) as rows,
            tc.tile_pool(name="osb", bufs=2) as osb,
            tc.tile_pool(name="fpp", bufs=4) as fppp,
            tc.tile_pool(name="psS", bufs=2, space="PSUM") as psS,
            tc.tile_pool(name="psOV", bufs=1, space="PSUM") as psOV,
            tc.tile_pool(name="psMM", bufs=2, space="PSUM") as psMM,
        ):
            # ---- constants / weights (SP queue) ----
            w_sb = {}
            for nm, d_ in (("wq", wq_d), ("wk", wk_d), ("wv", wv_d), ("wo", wo_d)):
                w_sb[nm] = cw.tile([P, 4, D], bf16, name=nm)
            wdr = {"wq": wq_d, "wk": wk_d, "wv": wv_d, "wo": wo_d}

            def w_chunk(nm, dc):
                nc.sync.dma_start(
                    w_sb[nm][:, :, dc * P:(dc + 1) * P],
                    wdr[nm].rearrange("(co p) d -> p co d", p=P)[:, :, dc * P:(dc + 1) * P])
            mk_sb = cst.tile([P, NJT], f32, name="mk_sb")
            mqi_r = cst.tile([1, NI], bf16, name="mqi_r")
            ident = cst.tile([P, P], bf16, name="ident")
            bo_f = cst.tile([1, D], f32, name="bo_f")
            ones_r = cst.tile([1, P], bf16, name="ones_r")
            nc.vector.memset(ones_r[:], 1.0)
            bo_hi = cst.tile([1, D], bf16, name="bo_hi")

            for rep in range(reps):
                # ---- x / xq loads (already transposed on host) ----
                xT = xp.tile([P, 4, N], bf16, name="xT")
                xqT = xp.tile([P, 4, NI], bf16, name="xqT")

                def tr_x(lo, hi, eng=None):
                    # gpsimd DGE queue: parallel with the SP weight loads
                    (eng or nc.gpsimd).dma_start(xT[:, :, lo:hi], xT_d[:, :, lo:hi])

                def tr_xq(lo, hi):
                    # scalar (ACT) DGE queue, idle during startup
                    nc.scalar.dma_start(xqT[:, :, lo:hi], xqT_d[:, :, lo:hi])

                # critical first quarter (cc=0 slices) lands before the rest
                # so the first matmul of kt_group(0,0)/qt_group(0,0) can
                # start ~4x sooner
                wkr = wdr["wk"].rearrange("(co p) d -> p co d", p=P)
                wqr = wdr["wq"].rearrange("(co p) d -> p co d", p=P)
                nc.sync.dma_start(w_sb["wk"][:, 0, 0:P], wkr[:, 0, 0:P])
                nc.gpsimd.dma_start(xT[:, 0, 0:512], xT_d[:, 0, 0:512])
                nc.sync.dma_start(w_sb["wq"][:, 0, 0:P], wqr[:, 0, 0:P])
                nc.scalar.dma_start(xqT[:, 0, 0:512], xqT_d[:, 0, 0:512])
                nc.sync.dma_start(w_sb["wk"][:, 1:4, 0:P], wkr[:, 1:4, 0:P])
                nc.gpsimd.dma_start(xT[:, 1:4, 0:512], xT_d[:, 1:4, 0:512])
                nc.sync.dma_start(w_sb["wq"][:, 1:4, 0:P], wqr[:, 1:4, 0:P])
                nc.scalar.dma_start(xqT[:, 1:4, 0:512], xqT_d[:, 1:4, 0:512])
                for dc in range(4):
                    w_chunk("wv", dc)
                nc.sync.dma_start(mk_sb[:], mk_d[:, :])
                tr_x(512, 1024)
                tr_x(1024, 2048)
                for dc in range(1, 4):
                    w_chunk("wk", dc)
                    w_chunk("wq", dc)
                nc.sync.dma_start(ident[:], id_d[:, :])
                nc.sync.dma_start(mqi_r[:], mqi_d[None, :])
                tr_xq(512, 1024)
                for dc in range(4):
                    w_chunk("wo", dc)
                nc.sync.dma_start(bo_f[:], bo_d[None, :])
                nc.gpsimd.tensor_copy(bo_hi[:], bo_f[:])

                kT = kqv.tile([P, 4, N], bf16, name="kT")
                qT = kqv.tile([P, 4, NI], bf16, name="qT")
                v_sb = kqv.tile([P, NJT, H * 65], bf16, name="v_sb")
                v65 = v_sb[:].rearrange("p jo (h e) -> p jo h e", e=65)
                oT = kqv.tile([P, 4, NI], bf16, name="oT")

                # ---- emit-closures for projection work (interleavable) ----
                def _drain(dst, ps, on_act):
                    # kT/qT PSUM drains alternate ACT/DVE: ACT has idle
                    # capacity during the projection phase
                    if on_act:
                        nc.scalar.activation(dst, ps, AF.Copy)
                    else:
                        nc.vector.tensor_copy(dst, ps)

                def kt_group(dc, nt):
                    ps = psMM.tile([P, 512], f32, name="mm")
                    for cc in range(4):
                        nc.tensor.matmul(
                            ps[:], w_sb["wk"][:, cc, dc * P:(dc + 1) * P],
                            xT[:, cc, nt * 512:(nt + 1) * 512],
                            start=(cc == 0), stop=(cc == 3))
                    _drain(kT[:, dc, nt * 512:(nt + 1) * 512], ps[:],
                           (dc + nt) % 2 == 0)

                def qt_group(dc, nt):
                    ps = psMM.tile([P, 512], f32, name="mm")
                    for cc in range(4):
                        nc.tensor.matmul(
                            ps[:], w_sb["wq"][:, cc, dc * P:(dc + 1) * P],
                            xqT[:, cc, nt * 512:(nt + 1) * 512],
                            start=(cc == 0), stop=(cc == 3))
                    _drain(qT[:, dc, nt * 512:(nt + 1) * 512], ps[:],
                           (dc + nt) % 2 == 1)

                def v_group(jt):
                    ps = psMM.tile([P, 512], f32, name="mm")
                    for cc in range(4):
                        nc.tensor.matmul(
                            ps[:], xT[:, cc, jt * P:(jt + 1) * P],
                            w_sb["wv"][:, cc, :],
                            start=(cc == 0), stop=(cc == 3))
                    # rows of masked keys -> 0 (mask folded into the drain).
                    # Odd jt drain via ACT (Copy with per-partition scale),
                    # even jt via DVE — balances the two PSUM-drain engines.
                    if jt % 2 == 1:
                        nc.scalar.activation(
                            v65[:, jt, :, 0:64],
                            ps[:].rearrange("p (h dd) -> p h dd", h=H),
                            AF.Copy, scale=mk_sb[:, jt:jt + 1])
                    else:
                        nc.vector.tensor_scalar(
                            v65[:, jt, :, 0:64],
                            ps[:].rearrange("p (h dd) -> p h dd", h=H),
                            mk_sb[:, jt:jt + 1], None, OP.mult)
                    # the "[V|mask]" column: denominator counts unmasked keys
                    nc.gpsimd.tensor_copy(
                        v65[:, jt, :, 64],
                        mk_sb[:, jt:jt + 1].to_broadcast((P, H)))

                meanrow = cst.tile([1, H, 65], bf16, name="meanrow")
                mxT = cst.tile([P, 4], bf16, name="mxT")

                def mean_fin():
                    # meanrow65[h] = [mean_n(x) @ Wv slice | 1.0]: the x-mean
                    # comes precomputed from host marshaling (mask NOT
                    # applied: the reference's uniform fallback averages all
                    # keys).
                    nc.sync.dma_start(mxT[:], mx_d[:, :])
                    mv_ps = psMM.tile([1, D], f32, name="mm")
                    for cc in range(4):
                        nc.tensor.matmul(mv_ps[:], mxT[:, cc:cc + 1],
                                         w_sb["wv"][:, cc, :],
                                         start=(cc == 0), stop=(cc == 3))
                    nc.vector.memset(meanrow[0:1, :, 64], 1.0)
                    nc.vector.tensor_copy(
                        meanrow[0:1, :, 0:64],
                        mv_ps[:].rearrange("o (h dd) -> o h dd", h=H))

                # split o-projection: 3/4 of the
                # contraction + bias runs during the last block; only the
                # hp=3 slice waits for the final attention block
                fpp_tiles = {}

                def oproj_part1(ic):
                    fp = psMM.tile([P, 512], f32, name="mm")
                    for cc in range(3):
                        nc.tensor.matmul(fp[:], oT[:, cc, ic * P:(ic + 1) * P],
                                         w_sb["wo"][:, cc, :],
                                         start=(cc == 0), stop=False)
                    nc.tensor.matmul(fp[:], ones_r[0:1, :], bo_hi[:],
                                     start=False, stop=True)
                    t = fppp.tile([P, 512], f32, name="fpp")
                    nc.vector.tensor_copy(t[:], fp[:])
                    fpp_tiles[ic] = t

                def oproj_part2(ic):
                    fp = psMM.tile([P, 512], f32, name="mm")
                    nc.tensor.matmul(fp[:], oT[:, 3, ic * P:(ic + 1) * P],
                                     w_sb["wo"][:, 3, :], start=True, stop=True)
                    o = osb.tile([P, 512], f32, name="o")
                    nc.vector.tensor_tensor(o[:], fp[:], fpp_tiles[ic][:], OP.add)
                    nc.sync.dma_start(out_d[ic * P:(ic + 1) * P, :], o[:])

                # ---- emission schedule ----
                # upfront: minimum to start S(hp0,it2=0,jt0) quickly
                bg = []
                kt_group(0, 0)
                qt_group(0, 0)
                v_group(0)
                v_group(1)
                # bg: just-in-time order; slots 0..11 pop two items each.
                # All 8 mean half-reduces sit BEHIND the v drains so the DVE
                # FIFO is clear during the early PV-gating window.
                for item in [
                    (v_group, (2,)), (v_group, (3,)),
                    (kt_group, (0, 1)), (v_group, (4,)),
                    (v_group, (5,)), (kt_group, (0, 2)),
                    (kt_group, (0, 3)), (v_group, (6,)),
                    (v_group, (7,)), (v_group, (8,)),
                    (v_group, (9,)), (v_group, (10,)),
                    (v_group, (11,)), (v_group, (12,)),
                    (v_group, (13,)), (v_group, (14,)),
                    (v_group, (15,)),
                    (mean_fin, ()),
                    (qt_group, (1, 0)),
                    (kt_group, (1, 0)), (kt_group, (1, 1)),
                    (kt_group, (1, 2)), (kt_group, (1, 3)),
                    None, (qt_group, (2, 0)), None,
                    (kt_group, (2, 0)), None, (kt_group, (2, 1)),
                    None, (kt_group, (2, 2)), None, (kt_group, (2, 3)),
                    None, (qt_group, (3, 0)), None,
                    (kt_group, (3, 0)), None, (kt_group, (3, 1)),
                    (kt_group, (3, 2)), (kt_group, (3, 3)),
                    None, (qt_group, (0, 1)), None, (qt_group, (1, 1)),
                    None, (qt_group, (2, 1)), None, (qt_group, (3, 1)),
                ]:
                    bg.append(item)

                # ---- attention ----
                slot = [0]
                deferred = [None]

                def norm_block(ovA, ovB, hp, it2, h0, h1):
                    state = {}

                    def emit():
                        # masked-query fallback: ov += mqi (x) meanrow (also
                        # biases the denominator column by mqi)
                        for ov, h in ((ovA, h0), (ovB, h1)):
                            for qb in range(4):
                                q0 = it2 * 512 + qb * P
                                nc.tensor.matmul(
                                    ov[:, qb, :], mqi_r[0:1, q0:q0 + P],
                                    meanrow[0:1, h, :],
                                    start=False, stop=(qb == 3),
                                    skip_group_check=True)
                        # normalize + transpose back to [d, q]
                        norm = []
                        for ov in (ovA, ovB):
                            rc = rows.tile([P, 4], f32, name="rc")
                            nc.vector.reciprocal(rc[:], ov[:, :, 64])
                            ob = obf.tile([P, 4, 64], bf16, name="ob")
                            nc.vector.tensor_copy(ob[:], ov[:, :, 0:64])
                            norm.append((rc, ob))
                        dgs = []
                        for rc, ob in norm:
                            for qb in range(4):
                                dg = dgp.tile([P, P], bf16, name="dg")
                                # SBUF->SBUF diag scale: run on gpsimd to
                                # keep DVE free for the exp share
                                nc.gpsimd.tensor_scalar(
                                    dg[:], ident[:], rc[:, qb:qb + 1], None,
                                    OP.mult)
                                dgs.append(dg)
                        state["norm"] = norm
                        state["dgs"] = dgs

                    def emit2():
                        norm, dgs = state["norm"], state["dgs"]
                        for qb in range(4):
                            tp = psMM.tile([P, P], f32, name="mm")
                            for tag, (rc, ob) in enumerate(norm):
                                nc.tensor.matmul(
                                    tp[64 * tag:64 * tag + 64, :], ob[:, qb, :],
                                    dgs[4 * tag + qb][:], start=True, stop=True,
                                    tile_position=(0, 64 * tag))
                            nc.vector.tensor_copy(
                                oT[:, hp,
                                   it2 * 512 + qb * P:it2 * 512 + (qb + 1) * P],
                                tp[:])
                    return emit, emit2

                for it2 in range(2):
                    for hp in range(4):
                        if hp == 3:
                            # 3/4 of this it2's o-projection can run as soon
                            # as hp=2's transposes land
                            ics = (0, 1, 2, 3) if it2 == 0 else (4, 5, 6, 7)
                            bg.extend([None, None, None,
                                       (oproj_part1, (ics[0],)), None,
                                       (oproj_part1, (ics[1],)), None,
                                       (oproj_part1, (ics[2],)), None,
                                       (oproj_part1, (ics[3],))])
                        if it2 == 1 and hp == 0:
                            # the cc=3 remainder of it2=0's o-projection:
                            # ready once (it2=0, hp=3)'s transposes (emitted
                            # at idx 2 of this block) land
                            bg.extend([None, None, None,
                                       (oproj_part2, (0,)), None,
                                       (oproj_part2, (1,)), None,
                                       (oproj_part2, (2,)), None,
                                       (oproj_part2, (3,))])
                        h0, h1 = 2 * hp, 2 * hp + 1
                        isl = slice(it2 * 512, (it2 + 1) * 512)
                        ovA = psOV.tile([P, 4, 65], f32, name="ovA")
                        ovB = psOV.tile([P, 4, 65], f32, name="ovB")
                        def pv(jt, pt):
                            for qb in range(4):
                                qsl = slice(qb * P, (qb + 1) * P)
                                nc.tensor.matmul(
                                    ovA[:, qb, :], pt[:, 0, qsl],
                                    v65[:, jt, h0, :],
                                    start=(jt == 0 and qb == 0), stop=False,
                                    skip_group_check=True)
                                nc.tensor.matmul(
                                    ovB[:, qb, :], pt[:, 1, qsl],
                                    v65[:, jt, h1, :],
                                    start=(jt == 0 and qb == 0), stop=False,
                                    skip_group_check=True)

                        # software-pipelined: S(jt)/exp(jt) issue ahead of
                        # PV(jt-1) so the next scores never queue behind the
                        # PV burst; bg items fill the exp wait.
                        prev_p = None
                        for jt in range(NJT):
                            s = psS.tile([P, 2, 512], f32, name="s")
                            nc.tensor.matmul(
                                s[:, 0, :], kT[0:64, hp, jt * P:(jt + 1) * P],
                                qT[0:64, hp, isl], start=True, stop=True,
                                tile_position=(0, 0))
                            nc.tensor.matmul(
                                s[:, 1, :], kT[64:128, hp, jt * P:(jt + 1) * P],
                                qT[64:128, hp, isl], start=True, stop=True,
                                tile_position=(64, 0))
                            p = pTp.tile([P, 2, 512], bf16, name="p")
                            if jt in dve_exp_jts:
                                nc.vector.tensor_scalar(
                                    p[:].bitcast(i16), s[:], EXP_A, EXP_B,
                                    OP.mult, OP.add)
                            else:
                                nc.scalar.activation(p[:], s[:], AF.Exp,
                                                     scale=0.125)
                            if jt == 0 and deferred[0] is not None:
                                deferred[0][0]()  # previous block: fold + rc/diag
                            if jt == 2 and deferred[0] is not None:
                                deferred[0][1]()  # previous block: transposes
                                deferred[0] = None
                            npop = 2 if slot[0] < 14 else 1
                            for _ in range(npop):
                                if bg:
                                    item = bg.pop(0)
                                    if item is not None:
                                        f, a = item
                                        f(*a)
                            slot[0] += 1
                            if prev_p is not None:
                                pv(jt - 1, prev_p)
                            prev_p = p
                        pv(NJT - 1, prev_p)
                        deferred[0] = norm_block(ovA, ovB, hp, it2, h0, h1)


                e1, e2 = deferred[0]
                e1()
                e2()
                deferred[0] = None
                while bg:
                    item = bg.pop(0)
                    if item is not None:
                        f, a = item
                        f(*a)
                for ic in range(4, 8):
                    oproj_part2(ic)

    nc.compile()
    return nc


def make_in_maps(x, mask_k, mask_q, Wq, Wk, Wv, Wo, bo, reps=1):
    """Shard full inputs into 8 per-core input maps (host-side marshaling)."""
    bf = ml_dtypes.bfloat16
    x_bf = x.astype(bf)
    # host-side transpose into the on-chip layout [p, d_chunk, n]
    xT_all = np.ascontiguousarray(
        x_bf.transpose(0, 2, 1).reshape(B, 4, P, N).transpose(0, 2, 1, 3))
    w = {"Wq": Wq.astype(bf), "Wk": Wk.astype(bf), "Wv": Wv.astype(bf),
         "Wo": Wo.astype(bf), "bo": bo.astype(np.float32)}
    ident = np.eye(P, dtype=bf)
    in_maps = []
    for c in range(8):
        b, hf = c // 2, c % 2
        qsl = slice(hf * NI, (hf + 1) * NI)
        mk = mask_k[b].astype(np.float32)
        mq = mask_q[b, qsl].astype(np.float32)
        in_maps.append({
            "pad": np.zeros((reps, 2), np.float32),
            "xT": xT_all[b],
            "xqT": np.ascontiguousarray(xT_all[b][:, :, qsl]),
            "mx": np.ascontiguousarray(
                x[b].mean(axis=0, dtype=np.float64).astype(bf)
                .reshape(4, P).T),
            "mk": np.ascontiguousarray(mk.reshape(NJT, P).T),
            "mqi": ((1.0 - mq) * 1e30).astype(bf),
            "ident": ident,
            **w,
        })
    return in_maps


def assemble_out(results):
    out = np.empty((B, N, D), dtype=np.float32)
    for c in range(8):
        b, hf = c // 2, c % 2
        out[b, hf * NI:(hf + 1) * NI, :] = results[c]["out"]
    return out


_NC_CACHE = {}


def kernel(x, mask_k, mask_q, Wq, Wk, Wv, Wo, bo):
    from concourse.bass_utils import run_bass_kernel_spmd

    x = np.asarray(x, dtype=np.float32)
    mask_k = np.asarray(mask_k)
    mask_q = np.asarray(mask_q)
    Wq = np.asarray(Wq, dtype=np.float32)
    Wk = np.asarray(Wk, dtype=np.float32)
    Wv = np.asarray(Wv, dtype=np.float32)
    Wo = np.asarray(Wo, dtype=np.float32)
    bo = np.asarray(bo, dtype=np.float32)

    if "nc" not in _NC_CACHE:
        _NC_CACHE["nc"] = build_nc(reps=1)
    nc = _NC_CACHE["nc"]
    in_maps = make_in_maps(x, mask_k, mask_q, Wq, Wk, Wv, Wo, bo, reps=1)
    res = run_bass_kernel_spmd(nc, in_maps, core_ids=list(range(8)))
    return assemble_out(res.results)

